# revision 13
# baseline (speedup 1.0000x reference)
"""Trainium2 Bass kernel for nn_MetapopLayer (metapopulation SIR scan).

Math: per sample n (1024 total), M=64 locations, C=4 compartments, 100 steps:
    p[n,i]   = 1 - exp(sum_j log(1 - beta*rho[n,i,1]*Rt[n,i,j]/ntot[n,j]))
    q        = R @ p          (per-sample 64x64 matvec)
    new_inf  = (1 - sum_c rho) * q
    rho'     = rho @ T + e0*new_inf, clipped to [0, 1e10]
    trajectory records pre-update rho.

Key device trick: |beta*rho1*Rt/ntot| <= ~0.006, so
p(a) = 1 - exp(-sum_m a^m P_m/m)  (a = rho[n,i,1]) is replaced by a degree-D
polynomial  p(a) = sum_d c_d[n,i] a^d  with coefficients precomputed on host
in float64 (exact to ~1e-10, far below fp32 noise).  The device step is then
pure fp32 tensor ops: Horner (11 small DVE ops), a broadcast-mul + grouped
reduce for the matvec, and a broadcast-mul + grouped reduce for rho@T.

Sharding: pure data-parallel over samples; 128 samples per core on the 128
SBUF partitions.  Raw Bass (Block) implementation — the Tile context's tail
drain trips a sync-wait limit in this walrus build, so semaphores are manual.
"""
import numpy as np

import concourse.bass as bass
from concourse import mybir
from concourse.bass_utils import run_bass_kernel_spmd

F32 = mybir.dt.float32
N, M, C = 1024, 64, 4
TIMESTEPS = 100
NCORES = 8
NS = N // NCORES            # 128 samples per core = SBUF partitions
DEG = 6                     # polynomial degree for p(a)
CLIP_MAX = 1e10


# ----------------------------------------------------------------------
# host-side precompute: polynomial coefficients c_d[n,i]
# ----------------------------------------------------------------------
def _precompute_coeffs(R, beta):
    R64 = R.astype(np.float64)
    ntot = R64.sum(axis=1)                                   # (N, M)
    Rt = np.transpose(R64).reshape(N, M, M)                  # faithful reshape
    V = beta.astype(np.float64)[:, None, None] * Rt / ntot[:, None, :]

    DEG_I = 12   # internal composition degree
    # g(a) = sum_m (P_m/m) a^m
    G = np.zeros((DEG_I + 1, N, M))
    Vp = np.ones_like(V)
    for m in range(1, DEG_I + 1):
        Vp = Vp * V
        G[m] = Vp.sum(axis=2) / m
    # E = exp(-g) as truncated power series;  p = 1 - E
    E = np.zeros((DEG_I + 1, N, M))
    E[0] = 1.0
    Gj = np.zeros((DEG_I + 1, N, M)); Gj[0] = 1.0
    fact = 1.0
    for j in range(1, DEG_I + 1):
        new = np.zeros_like(Gj)
        for d1 in range(j - 1, DEG_I + 1):
            if not Gj[d1].any():
                continue
            for d2 in range(1, DEG_I + 1 - d1):
                new[d1 + d2] += Gj[d1] * G[d2]
        Gj = new
        fact *= j
        E += ((-1) ** j) * Gj / fact
    Cc = -E
    Cc[0] = 0.0
    return Cc[1 : DEG + 1].astype(np.float32)                # (DEG, N, M)


# ----------------------------------------------------------------------
# device kernel builder (per-core program, SPMD across 8 cores)
# ----------------------------------------------------------------------
def _build_bass(run_steps=TIMESTEPS):
    nc = bass.Bass()
    R_d = nc.dram_tensor("R", [NS, M * M], F32, kind="ExternalInput")     # (n,(i,k))
    cd_d = nc.dram_tensor("cd", [NS, DEG * M], F32, kind="ExternalInput")  # (n,(d,i))
    Tb_d = nc.dram_tensor("Tb", [NS, 16], F32, kind="ExternalInput")       # (n,(k,l))
    rho0_d = nc.dram_tensor("rho0", [NS, M * C], F32, kind="ExternalInput")
    traj_d = nc.dram_tensor("traj", [TIMESTEPS, NS, M * C], F32,
                            kind="ExternalOutput")

    mult, add_, mx = mybir.AluOpType.mult, mybir.AluOpType.add, mybir.AluOpType.max

    with (
        nc.sbuf_tensor("R_t", [NS, M * M], F32) as R_t,
        nc.sbuf_tensor("cd_t", [NS, DEG * M], F32) as cd_t,
        nc.sbuf_tensor("Tb_t", [NS, 16], F32) as Tb_t,
        nc.sbuf_tensor("rhoA", [NS, M * C], F32) as rhoA,
        nc.sbuf_tensor("rhoB", [NS, M * C], F32) as rhoB,
        nc.sbuf_tensor("t_mv", [NS, M * M], F32) as t_mv,
        nc.sbuf_tensor("Gm", [NS, M * 16], F32) as Gm,
        nc.sbuf_tensor("h_t", [NS, M], F32) as h_t,
        nc.sbuf_tensor("p_t", [NS, M], F32) as p_t,
        nc.sbuf_tensor("q_t", [NS, M], F32) as q_t,
        nc.sbuf_tensor("sr_t", [NS, M], F32) as sr_t,
        nc.sbuf_tensor("u_t", [NS, M], F32) as u_t,
        nc.sbuf_tensor("ni_t", [NS, M], F32) as ni_t,
        nc.sbuf_tensor("ones_t", [NS, M], F32) as ones_t,
        nc.sbuf_tensor("zero_t", [NS, M], F32) as zero_t,
        nc.semaphore("s_in") as s_in,       # input DMAs done (16 each)
        nc.semaphore("s_state") as s_state,  # DVE step counter
        nc.semaphore("s_out") as s_out,     # traj out-DMAs (16 each)
        nc.semaphore("s_gm") as s_gm,       # GPSIMD G-mul per step
        nc.Block() as block,
    ):
        rho = [rhoA, rhoB]

        def rho_ap(buf, view):
            base = buf[:].ap[0]
            if view == "a":       # rho[:, 1::4]  (= compartment 1, per i)
                return bass.AP(buf, 1, [base, [4, M]])
            if view == "col0":    # rho[:, 0::4]
                return bass.AP(buf, 0, [base, [4, M]])
            if view == "ic":      # (i, c) for srho reduce
                return bass.AP(buf, 0, [base, [4, M], [1, 4]])
            if view == "G_in":    # (i, l, k): rho[n, i*4+k] bcast over l
                return bass.AP(buf, 0, [base, [4, M], [0, 4], [1, 4]])
            raise ValueError(view)

        @block.sync
        def _(sync):
            sync.dma_start(R_t[:], R_d[:, :]).then_inc(s_in, 16)
            sync.dma_start(cd_t[:], cd_d[:, :]).then_inc(s_in, 16)
            sync.dma_start(Tb_t[:], Tb_d[:, :]).then_inc(s_in, 16)
            sync.dma_start(rhoA[:], rho0_d[:, :]).then_inc(s_in, 16)
            sync.wait_ge(s_in, 64)                  # inputs landed
            for t in range(run_steps):
                sync.wait_ge(s_state, t)            # rho_t finalized
                src = rho[t % 2]
                dst = bass.AP(traj_d, t * NS * M * C,
                              [[M * C, NS], [1, M * C]])
                sync.dma_start(dst, src[:]).then_inc(s_out, 16)
            sync.wait_ge(s_out, 16 * run_steps)     # all outputs landed

        @block.gpsimd
        def _(gpsimd):
            # G-mul for step t: Gm[n,(i,l,k)] = rho_t[n,(i,k)] * T[n,(k,l)]
            Tb_bc = bass.AP(Tb_t, 0, [Tb_t[:].ap[0], [0, M], [1, 4], [4, 4]])
            Gm_v = Gm[:].rearrange("n (i l k) -> n i l k", i=M, l=4)
            gpsimd.wait_ge(s_in, 64)
            for t in range(run_steps):
                if t > 0:
                    gpsimd.wait_ge(s_state, t)      # rho_t ready + prev Gm read
                gpsimd.tensor_tensor(out=Gm_v, in0=rho_ap(rho[t % 2], "G_in"),
                                     in1=Tb_bc, op=mult).then_inc(s_gm, 1)

        @block.vector
        def _(vector):
            R_ik = R_t[:].rearrange("n (i k) -> n i k", i=M)
            t_ik = t_mv[:].rearrange("n (i k) -> n i k", i=M)
            p_bc = bass.AP(p_t, 0, [p_t[:].ap[0], [0, M], [1, M]])
            Gm_red = Gm[:].rearrange("n (il k) -> n il k", k=4)
            sub = mybir.AluOpType.subtract
            vector.memset(ones_t[:], 1.0)
            vector.memset(zero_t[:], 0.0)
            vector.wait_ge(s_in, 64)
            for t in range(run_steps):
                cur, nxt = rho[t % 2], rho[(t + 1) % 2]
                a_v = rho_ap(cur, "a")
                # srho, u = 1 - srho (early: consumed several ops later)
                vector.tensor_reduce(out=sr_t[:], in_=rho_ap(cur, "ic"),
                                     axis=mybir.AxisListType.X, op=add_)
                vector.tensor_tensor(out=u_t[:], in0=ones_t[:], in1=sr_t[:], op=sub)
                # p = Horner(c, a)
                vector.tensor_tensor(out=h_t[:], in0=cd_t[:, (DEG - 1) * M : DEG * M],
                                     in1=a_v, op=mult)
                for d in range(DEG - 1, 0, -1):
                    vector.tensor_tensor(out=h_t[:], in0=h_t[:],
                                         in1=cd_t[:, (d - 1) * M : d * M], op=add_)
                    if d > 1:
                        vector.tensor_tensor(out=h_t[:], in0=h_t[:], in1=a_v,
                                             op=mult)
                vector.tensor_tensor(out=p_t[:], in0=h_t[:], in1=a_v, op=mult)
                # q = R @ p  (broadcast-mul + grouped reduce)
                vector.tensor_tensor(out=t_ik, in0=R_ik, in1=p_bc, op=mult)
                vector.tensor_reduce(out=q_t[:], in_=t_ik,
                                     axis=mybir.AxisListType.X, op=add_)
                vector.tensor_tensor(out=ni_t[:], in0=u_t[:], in1=q_t[:], op=mult)
                # rho_next = rho @ T  (+ new_inf into c=0, clip)
                if t > 0:
                    vector.wait_ge(s_out, 16 * t)   # traj[t-1] DMA done
                vector.wait_ge(s_gm, t + 1)         # Gm ready
                vector.tensor_reduce(out=nxt[:], in_=Gm_red,
                                     axis=mybir.AxisListType.X, op=add_)
                col0 = rho_ap(nxt, "col0")
                vector.tensor_tensor(out=col0, in0=col0, in1=ni_t[:], op=add_)
                vector.tensor_tensor(out=col0, in0=col0, in1=zero_t[:],
                                     op=mx).then_inc(s_state, 1)
    return nc


_NC_CACHE = None


def kernel(R, T, rho0, beta):
    global _NC_CACHE
    R = np.ascontiguousarray(R, np.float32)
    T = np.ascontiguousarray(T, np.float32)
    rho0 = np.ascontiguousarray(rho0, np.float32)
    beta = np.ascontiguousarray(beta, np.float32)

    cd = _precompute_coeffs(R, beta)                          # (DEG, N, M)
    cd_dev = np.ascontiguousarray(cd.transpose(1, 0, 2)).reshape(N, DEG * M)

    if _NC_CACHE is None:
        _NC_CACHE = _build_bass()
    nc = _NC_CACHE

    in_maps = []
    for c in range(NCORES):
        s = slice(c * NS, (c + 1) * NS)
        in_maps.append({
            "R": R[s].reshape(NS, M * M),
            "cd": cd_dev[s],
            "Tb": T[s].reshape(NS, 16),
            "rho0": rho0[s].reshape(NS, M * C),
        })
    res = run_bass_kernel_spmd(nc, in_maps, core_ids=list(range(NCORES)))
    parts = [r["traj"].reshape(TIMESTEPS, NS, M, C) for r in res.results]
    return np.concatenate(parts, axis=1)
